# revision 18
# baseline (speedup 1.0000x reference)
"""Bahdanau attention Trainium2 Bass kernel.

Computation (per batch row b):
    q_proj = query @ Wa_w.T + Wa_b                     # [1, H]
    k_proj = keys  @ Ua_w.T + Ua_b                     # [S, H]
    e      = tanh(q_proj + k_proj)                     # [S, H]
    scores = e @ Va_w.T (+ Va_b, dropped: softmax-invariant)
    weights = softmax(scores)                          # [1, S]
    context = weights @ keys                           # [1, H]

Sharding: data-parallel over batch (32 rows) across 8 cores, 4 rows/core;
weights (Wa/Ua/Va) replicated per core. Host-side sharding re-lays-out the
operands into the layouts the PE needs (keys/Ua/Wa/Va pre-transposed) and
casts the matmul operands to MM_MODE precision, so the device spends no
TensorE cycles on transposes and no DVE cycles on casts.

Per-core dataflow (layout "B": o on partitions so the q_proj+bias add fuses
into the tanh ACT op as a per-partition bias, and scores come out
s-contiguous for the softmax):
  - keysT row-slabs [128(h), 2048(s)] DMA'd straight from DRAM; issue split
    across the SP and Pool sequencers.
  - k_projT[o, s] = UaT.T @ keysT (fp32 accumulate in PSUM).
  - e = tanh(k_projT + bias[o]) on ScalarE.
  - scores[1, s] += VaT[o,1].T @ e accumulated over the 8 o-tiles.
  - softmax on [1, 2048]: DVE reduce_max(negate)/reduce_sum/reciprocal,
    ScalarE exp.
  - weights row transposed to [s,1] columns via tiny PE transposes; context
    [1, h] += wT.T @ keys_nat[s, h] over 16 s-tiles (keys_nat is a second,
    natural-layout DMA stream).

MM_MODE picks the matmul operand dtype:
  "bf16" — fastest: half DMA bytes, ~1e-3..4e-3 rel error.
  "f32r" — fp32 storage, PE-internal rounding: ~1.5e-4 rel error, slower.
"""

import numpy as np
import ml_dtypes
from contextlib import ExitStack

import concourse.bass as bass
import concourse.tile as tile
from concourse import bacc, masks, mybir
from concourse import bass_utils

P = 128
H = 1024
S = 2048
B = 32
NCORES = 8
BC = B // NCORES          # batch rows per core
HT = H // P               # 8 h-tiles
OT = H // P               # 8 o-tiles
SC = 512                  # s-chunk (matmul moving free dim)
NCH = S // SC             # 4 chunks per row

F32 = mybir.dt.float32
F32R = mybir.dt.float32r
BF16 = mybir.dt.bfloat16
AF = mybir.ActivationFunctionType
AX = mybir.AxisListType

MM_MODE = "f32r"
MMDT = BF16 if MM_MODE == "bf16" else F32R
NPDT = ml_dtypes.bfloat16 if MM_MODE == "bf16" else np.float32


def build_tile_kernel(ctx: ExitStack, tc: tile.TileContext, io: dict,
                      repeats: int = 1):
    nc = tc.nc

    # ---- pools ----
    const = ctx.enter_context(tc.tile_pool(name="const", bufs=1))
    wpool = ctx.enter_context(tc.tile_pool(name="wts", bufs=1))
    # natural keys tiles [128, 1024] (context operand)
    keys_pool = ctx.enter_context(tc.tile_pool(name="keys", bufs=17))
    # keysT row-slabs [128(h), 2048(s)]: 8 per row + prefetch slack
    kt_pool = ctx.enter_context(tc.tile_pool(name="kt", bufs=12))
    e_pool = ctx.enter_context(tc.tile_pool(name="e", bufs=4))
    row_pool = ctx.enter_context(tc.tile_pool(name="row", bufs=1))
    wt_pool = ctx.enter_context(tc.tile_pool(name="wt", bufs=20))
    kproj_ps = ctx.enter_context(tc.tile_pool(name="kproj", bufs=2, space="PSUM"))
    sc_ps = ctx.enter_context(tc.tile_pool(name="scps", bufs=2, space="PSUM"))
    sm_ps = ctx.enter_context(tc.tile_pool(name="smps", bufs=2, space="PSUM"))

    ident = const.tile([P, P], F32)
    masks.make_identity(nc, ident[:])

    env = dict(locals())
    for _rep in range(repeats):
        _one_pass(nc, io, env)


def _one_pass(nc, io, env):
    wpool = env["wpool"]; keys_pool = env["keys_pool"]
    kt_pool = env["kt_pool"]; e_pool = env["e_pool"]; row_pool = env["row_pool"]
    wt_pool = env["wt_pool"]; kproj_ps = env["kproj_ps"]
    sc_ps = env["sc_ps"]; sm_ps = env["sm_ps"]
    ident = env["ident"]
    kt_d = io["keys_t"]; k_d = io["keys"]; qt_d = io["query_t"]
    uat_d = io["ua_t"]; wat_d = io["wa_t"]; vat_d = io["va_t"]
    ab_d = io["ab_sum"]
    ctx_d = io["context"]; wts_d = io["weights"]

    # ============== prep: DMA pre-transposed weights ==============
    uat = [wpool.tile([P, H], MMDT, tag=f"uat{ht}", name=f"uat{ht}")
           for ht in range(HT)]
    for ht in range(HT):
        nc.sync.dma_start(uat[ht][:], uat_d[ht * P:(ht + 1) * P, :])
    # WaT tiles (transient, q_proj only): borrow kt-pool slots
    wat = [kt_pool.tile([P, H], F32, tag="kt", name=f"wat{ht}")
           for ht in range(HT)]
    for ht in range(HT):
        nc.gpsimd.dma_start(wat[ht][:], wat_d[ht * P:(ht + 1) * P, :])
    qt = [wpool.tile([P, BC], F32, tag=f"qt{ht}", name=f"qt{ht}")
          for ht in range(HT)]
    for ht in range(HT):
        nc.sync.dma_start(qt[ht][:], qt_d[ht * P:(ht + 1) * P, :])
    vat = [wpool.tile([P, 1], MMDT, tag=f"vat{ot}", name=f"vat{ot}")
           for ot in range(OT)]
    for ot in range(OT):
        nc.sync.dma_start(vat[ot][:], vat_d[ot * P:(ot + 1) * P, :])
    bias_col = [wpool.tile([P, 1], F32, tag=f"bc{ot}", name=f"bc{ot}")
                for ot in range(OT)]
    for ot in range(OT):
        nc.sync.dma_start(bias_col[ot][:], ab_d[ot * P:(ot + 1) * P, :])

    # q_projT[ot] [128, BC] + bias column -> tanh bias vectors
    biases = [wpool.tile([P, BC], F32, tag=f"bias{ot}", name=f"bias{ot}")
              for ot in range(OT)]
    for ot in range(OT):
        pq = sm_ps.tile([P, BC], F32, tag="smps", name="pq")
        for ht in range(HT):
            nc.tensor.matmul(
                pq[:],
                lhsT=wat[ht][:, ot * P:(ot + 1) * P],
                rhs=qt[ht][:],
                start=(ht == 0),
                stop=(ht == HT - 1),
            )
        nc.vector.tensor_scalar_add(biases[ot][:], pq[:], bias_col[ot][:])

    # =====================  main loop over batch rows  =====================
    for b in range(BC):
        scores_sb = row_pool.tile([1, S], F32, tag="scores", name="scores_sb")
        # keysT row-slabs [128(h), 2048(s)]: one DMA per h-tile, split
        # across two sequencers
        kts = []  # kts[hh][ht] covers s half hh
        for hh in range(2):
            row = []
            for ht in range(HT):
                kt = kt_pool.tile([P, S // 2], MMDT, tag="kt", name="kt")
                eng = nc.sync if ht % 2 == 0 else nc.gpsimd
                eng.dma_start(
                    kt[:],
                    kt_d[b, ht * P:(ht + 1) * P,
                         hh * (S // 2):(hh + 1) * (S // 2)],
                )
                row.append(kt)
            kts.append(row)
        # natural keys tiles for the context matmul
        keys_row = []
        for k in range(S // P):
            knat = keys_pool.tile([P, H], MMDT, tag="keys", name="knat")
            nc.gpsimd.dma_start(knat[:], k_d[b, k * P:(k + 1) * P, :])
            keys_row.append(knat)
        for jj in range(2):  # chunk pairs; kts[jj] holds this pair's slabs
            sps = [sc_ps.tile([1, SC], F32, tag="scps", name=f"spsum{c}")
                   for c in range(2)]
            for ot in range(OT):
                # 2-bank PSUM tile: both chunks of the pair accumulate here
                kp = kproj_ps.tile([P, 2 * SC], F32, tag="kproj", name="kp")
                for c in range(2):
                    for ht in range(HT):
                        nc.tensor.matmul(
                            kp[:, c * SC:(c + 1) * SC],
                            lhsT=uat[ht][:, ot * P:(ot + 1) * P],
                            rhs=kts[jj][ht][:, c * SC:(c + 1) * SC],
                            start=(ht == 0),
                            stop=(ht == HT - 1),
                        )
                # one tanh over both banks
                e_sb = e_pool.tile([P, 2 * SC], MMDT, tag="e", name="e_sb")
                nc.scalar.activation(
                    e_sb[:], kp[:], AF.Tanh, bias=biases[ot][:, b:b + 1]
                )
                for c in range(2):
                    nc.tensor.matmul(
                        sps[c][:],
                        lhsT=vat[ot][:],
                        rhs=e_sb[:, c * SC:(c + 1) * SC],
                        start=(ot == 0),
                        stop=(ot == OT - 1),
                        skip_group_check=True,
                    )
            for c in range(2):
                nc.vector.tensor_copy(
                    scores_sb[:, (jj * 2 + c) * SC:(jj * 2 + c + 1) * SC],
                    sps[c][:],
                )

        # ----- softmax over [1, S] -----
        negm = row_pool.tile([1, 1], F32, tag="negm", name="negm")
        nc.vector.reduce_max(negm[:], scores_sb[:], axis=AX.X, negate=True)
        ew = row_pool.tile([1, S], F32, tag="ew", name="ew")
        nc.scalar.activation(ew[:], scores_sb[:], AF.Exp, bias=negm[:])
        zsum = row_pool.tile([1, 1], F32, tag="z", name="zsum")
        nc.vector.reduce_sum(zsum[:], ew[:], axis=AX.X)
        rz = row_pool.tile([1, 1], F32, tag="rz", name="rz")
        nc.vector.reciprocal(rz[:], zsum[:])
        wrow = row_pool.tile([1, S], F32, tag="wrow", name="wrow")
        nc.vector.tensor_scalar_mul(wrow[:], ew[:], rz[:])
        nc.sync.dma_start(wts_d[b:b + 1, :], wrow[:])

        # ----- context = weights @ keys -----
        wts_sb = []
        for k in range(S // P):
            tp = sm_ps.tile([P, 1], F32, tag="smps", name="wtp")
            nc.tensor.transpose(
                tp[:], wrow[:, k * P:(k + 1) * P], ident[0:1, 0:1]
            )
            wt_sb = wt_pool.tile([P, 1], MMDT, tag="wt", name="wt_sb")
            nc.vector.tensor_copy(wt_sb[:], tp[:])
            wts_sb.append(wt_sb)
        for half in range(2):
            cps = sm_ps.tile([1, SC], F32, tag="smps", name="cps")
            for k in range(S // P):
                nc.tensor.matmul(
                    cps[:],
                    lhsT=wts_sb[k][:],
                    rhs=keys_row[k][:, half * SC:(half + 1) * SC],
                    start=(k == 0),
                    stop=(k == S // P - 1),
                    skip_group_check=True,
                )
            csb = row_pool.tile([1, SC], F32, tag=f"ctx{half}", name=f"csb{half}")
            nc.vector.tensor_copy(csb[:], cps[:])
            nc.sync.dma_start(ctx_d[b:b + 1, half * SC:(half + 1) * SC], csb[:])


_CACHED_NC = {}


def build_nc(repeats: int = 1):
    if repeats in _CACHED_NC:
        return _CACHED_NC[repeats]
    nc = bacc.Bacc(
        "TRN2", target_bir_lowering=False, debug=False, num_devices=NCORES
    )
    io = {
        "keys": nc.dram_tensor("keys", [BC, S, H], MMDT, kind="ExternalInput").ap(),
        "keys_t": nc.dram_tensor("keys_t", [BC, H, S], MMDT, kind="ExternalInput").ap(),
        "query_t": nc.dram_tensor("query_t", [H, BC], F32, kind="ExternalInput").ap(),
        "ua_t": nc.dram_tensor("ua_t", [H, H], MMDT, kind="ExternalInput").ap(),
        "wa_t": nc.dram_tensor("wa_t", [H, H], F32, kind="ExternalInput").ap(),
        "va_t": nc.dram_tensor("va_t", [H, 1], MMDT, kind="ExternalInput").ap(),
        "ab_sum": nc.dram_tensor("ab_sum", [H, 1], F32, kind="ExternalInput").ap(),
        "context": nc.dram_tensor("context", [BC, H], F32, kind="ExternalOutput").ap(),
        "weights": nc.dram_tensor("weights", [BC, S], F32, kind="ExternalOutput").ap(),
    }
    with tile.TileContext(nc) as tc:
        with ExitStack() as ctx:
            build_tile_kernel(ctx, tc, io, repeats=repeats)
    nc.compile()
    _CACHED_NC[repeats] = nc
    return nc


def shard_inputs(inputs):
    """Full inputs -> per-core in_maps.

    Sharding: batch across cores; weights replicated. The shard step also
    produces the transposed layouts / operand dtypes the kernel needs.
    """
    f = lambda x: np.ascontiguousarray(np.asarray(x), dtype=np.float32)
    g = lambda x: np.ascontiguousarray(np.asarray(x, dtype=NPDT))
    query = f(inputs["query"]).reshape(B, H)
    keys = f(inputs["keys"])
    keys_mm = g(keys)                                    # [B, S, H] cast
    keys_t = np.ascontiguousarray(keys.transpose(0, 2, 1).astype(NPDT))
    query_t = np.ascontiguousarray(query.T)              # [H, B]
    ua_t = g(f(inputs["Ua_w"]).T)
    wa_t = np.ascontiguousarray(f(inputs["Wa_w"]).T)
    va_t = g(f(inputs["Va_w"]).reshape(1, H).T)          # [H, 1]
    ab_sum = np.ascontiguousarray(
        (f(inputs["Wa_b"]) + f(inputs["Ua_b"])).reshape(H, 1)
    )
    shared = {"ua_t": ua_t, "wa_t": wa_t, "va_t": va_t, "ab_sum": ab_sum}
    in_maps = []
    for c in range(NCORES):
        sl = slice(c * BC, (c + 1) * BC)
        in_maps.append(
            {
                "keys": np.ascontiguousarray(keys_mm[sl]),
                "keys_t": np.ascontiguousarray(keys_t[sl]),
                "query_t": np.ascontiguousarray(query_t[:, sl]),
                **shared,
            }
        )
    return in_maps


def run(inputs, trace=False):
    """Run on 8 cores; returns ((context, weights), BassKernelResults)."""
    nc = build_nc()
    in_maps = shard_inputs(inputs)
    res = bass_utils.run_bass_kernel_spmd(
        nc, in_maps, list(range(NCORES)), trace=trace
    )
    ctxs = np.stack([r["context"] for r in res.results])  # [8, BC, H]
    wts = np.stack([r["weights"] for r in res.results])  # [8, BC, S]
    context = ctxs.reshape(B, 1, H).astype(np.float32)
    weights = wts.reshape(B, 1, S).astype(np.float32)
    return (context, weights), res


def kernel(**inputs):
    out, _ = run(inputs, trace=False)
    return out


# revision 19
# speedup vs baseline: 1.1272x; 1.1272x over previous
"""Bahdanau attention Trainium2 Bass kernel.

Computation (per batch row b):
    q_proj = query @ Wa_w.T + Wa_b                     # [1, H]
    k_proj = keys  @ Ua_w.T + Ua_b                     # [S, H]
    e      = tanh(q_proj + k_proj)                     # [S, H]
    scores = e @ Va_w.T (+ Va_b, dropped: softmax-invariant)
    weights = softmax(scores)                          # [1, S]
    context = weights @ keys                           # [1, H]

Sharding: data-parallel over batch (32 rows) across 8 cores, 4 rows/core;
weights (Wa/Ua/Va) replicated per core. Host-side sharding re-lays-out the
operands into the layouts the PE needs (keys/Ua/Wa/Va pre-transposed) and
casts the matmul operands to MM_MODE precision, so the device spends no
TensorE cycles on transposes and no DVE cycles on casts.

Per-core dataflow (layout "B": o on partitions so the q_proj+bias add fuses
into the tanh ACT op as a per-partition bias, and scores come out
s-contiguous for the softmax):
  - keysT row-slabs [128(h), 2048(s)] DMA'd straight from DRAM; issue split
    across the SP and Pool sequencers.
  - k_projT[o, s] = UaT.T @ keysT (fp32 accumulate in PSUM).
  - e = tanh(k_projT + bias[o]) on ScalarE.
  - scores[1, s] += VaT[o,1].T @ e accumulated over the 8 o-tiles.
  - softmax on [1, 2048]: DVE reduce_max(negate)/reduce_sum/reciprocal,
    ScalarE exp.
  - weights row transposed to [s,1] columns via tiny PE transposes; context
    [1, h] += wT.T @ keys_nat[s, h] over 16 s-tiles (keys_nat is a second,
    natural-layout DMA stream).

MM_MODE picks the matmul operand dtype:
  "bf16" — fastest: half DMA bytes, ~1e-3..4e-3 rel error.
  "f32r" — fp32 storage, PE-internal rounding: ~1.5e-4 rel error, slower.
"""

import numpy as np
import ml_dtypes
from contextlib import ExitStack

import concourse.bass as bass
import concourse.tile as tile
from concourse import bacc, masks, mybir
from concourse import bass_utils

P = 128
H = 1024
S = 2048
B = 32
NCORES = 8
BC = B // NCORES          # batch rows per core
HT = H // P               # 8 h-tiles
OT = H // P               # 8 o-tiles
SC = 512                  # s-chunk (matmul moving free dim)
NCH = S // SC             # 4 chunks per row

F32 = mybir.dt.float32
F32R = mybir.dt.float32r
BF16 = mybir.dt.bfloat16
AF = mybir.ActivationFunctionType
AX = mybir.AxisListType

MM_MODE = "f32r"
MMDT = BF16 if MM_MODE == "bf16" else F32R
NPDT = ml_dtypes.bfloat16 if MM_MODE == "bf16" else np.float32


def build_tile_kernel(ctx: ExitStack, tc: tile.TileContext, io: dict,
                      repeats: int = 1):
    nc = tc.nc

    # ---- pools ----
    const = ctx.enter_context(tc.tile_pool(name="const", bufs=1))
    wpool = ctx.enter_context(tc.tile_pool(name="wts", bufs=1))
    # natural keys tiles [128, 1024] (context operand)
    keys_pool = ctx.enter_context(tc.tile_pool(name="keys", bufs=17))
    # keysT row-slabs [128(h), 2048(s)]: 8 per row + prefetch slack
    kt_pool = ctx.enter_context(tc.tile_pool(name="kt", bufs=14))
    e_pool = ctx.enter_context(tc.tile_pool(name="e", bufs=8))
    row_pool = ctx.enter_context(tc.tile_pool(name="row", bufs=1))
    wt_pool = ctx.enter_context(tc.tile_pool(name="wt", bufs=20))
    kproj_ps = ctx.enter_context(tc.tile_pool(name="kproj", bufs=4, space="PSUM"))
    sc_ps = ctx.enter_context(tc.tile_pool(name="scps", bufs=2, space="PSUM"))
    sm_ps = ctx.enter_context(tc.tile_pool(name="smps", bufs=2, space="PSUM"))

    ident = const.tile([P, P], F32)
    masks.make_identity(nc, ident[:])

    env = dict(locals())
    for _rep in range(repeats):
        _one_pass(nc, io, env)


def _one_pass(nc, io, env):
    wpool = env["wpool"]; keys_pool = env["keys_pool"]
    kt_pool = env["kt_pool"]; e_pool = env["e_pool"]; row_pool = env["row_pool"]
    wt_pool = env["wt_pool"]; kproj_ps = env["kproj_ps"]
    sc_ps = env["sc_ps"]; sm_ps = env["sm_ps"]
    ident = env["ident"]
    kt_d = io["keys_t"]; k_d = io["keys"]; qt_d = io["query_t"]
    uat_d = io["ua_t"]; wat_d = io["wa_t"]; vat_d = io["va_t"]
    ab_d = io["ab_sum"]
    ctx_d = io["context"]; wts_d = io["weights"]

    # ============== prep: DMA pre-transposed weights ==============
    uat = [wpool.tile([P, H], MMDT, tag=f"uat{ht}", name=f"uat{ht}")
           for ht in range(HT)]
    for ht in range(HT):
        nc.sync.dma_start(uat[ht][:], uat_d[ht * P:(ht + 1) * P, :])
    # WaT tiles (transient, q_proj only): borrow kt-pool slots
    wat = [kt_pool.tile([P, H], F32, tag="kt", name=f"wat{ht}")
           for ht in range(HT)]
    for ht in range(HT):
        nc.gpsimd.dma_start(wat[ht][:], wat_d[ht * P:(ht + 1) * P, :])
    qt = [wpool.tile([P, BC], F32, tag=f"qt{ht}", name=f"qt{ht}")
          for ht in range(HT)]
    for ht in range(HT):
        nc.sync.dma_start(qt[ht][:], qt_d[ht * P:(ht + 1) * P, :])
    vat = [wpool.tile([P, 1], MMDT, tag=f"vat{ot}", name=f"vat{ot}")
           for ot in range(OT)]
    for ot in range(OT):
        nc.sync.dma_start(vat[ot][:], vat_d[ot * P:(ot + 1) * P, :])
    bias_col = [wpool.tile([P, 1], F32, tag=f"bc{ot}", name=f"bc{ot}")
                for ot in range(OT)]
    for ot in range(OT):
        nc.sync.dma_start(bias_col[ot][:], ab_d[ot * P:(ot + 1) * P, :])

    # q_projT[ot] [128, BC] + bias column -> tanh bias vectors
    biases = [wpool.tile([P, BC], F32, tag=f"bias{ot}", name=f"bias{ot}")
              for ot in range(OT)]
    for ot in range(OT):
        pq = sm_ps.tile([P, BC], F32, tag="smps", name="pq")
        for ht in range(HT):
            nc.tensor.matmul(
                pq[:],
                lhsT=wat[ht][:, ot * P:(ot + 1) * P],
                rhs=qt[ht][:],
                start=(ht == 0),
                stop=(ht == HT - 1),
            )
        nc.vector.tensor_scalar_add(biases[ot][:], pq[:], bias_col[ot][:])

    # =====================  main loop over batch rows  =====================
    for b in range(BC):
        scores_sb = row_pool.tile([1, S], F32, tag="scores", name="scores_sb")
        # keysT row-slabs [128(h), 2048(s)]: one DMA per h-tile, split
        # across two sequencers
        kts = []  # kts[hh][ht] covers s half hh
        for hh in range(2):
            row = []
            for ht in range(HT):
                kt = kt_pool.tile([P, S // 2], MMDT, tag="kt", name="kt")
                eng = nc.sync if ht % 2 == 0 else nc.gpsimd
                eng.dma_start(
                    kt[:],
                    kt_d[b, ht * P:(ht + 1) * P,
                         hh * (S // 2):(hh + 1) * (S // 2)],
                )
                row.append(kt)
            kts.append(row)
        # natural keys tiles for the context matmul
        keys_row = []
        for k in range(S // P):
            knat = keys_pool.tile([P, H], MMDT, tag="keys", name="knat")
            nc.gpsimd.dma_start(knat[:], k_d[b, k * P:(k + 1) * P, :])
            keys_row.append(knat)
        for j in range(NCH):
            spsum = sc_ps.tile([1, SC], F32, tag="scps", name="spsum")
            for ot in range(OT):
                kp = kproj_ps.tile([P, SC], F32, tag="kproj", name="kp")
                for ht in range(HT):
                    nc.tensor.matmul(
                        kp[:],
                        lhsT=uat[ht][:, ot * P:(ot + 1) * P],
                        rhs=kts[j // 2][ht][:, (j % 2) * SC:(j % 2 + 1) * SC],
                        start=(ht == 0),
                        stop=(ht == HT - 1),
                    )
                e_sb = e_pool.tile([P, SC], MMDT, tag="e", name="e_sb")
                nc.scalar.activation(
                    e_sb[:], kp[:], AF.Tanh, bias=biases[ot][:, b:b + 1]
                )
                nc.tensor.matmul(
                    spsum[:],
                    lhsT=vat[ot][:],
                    rhs=e_sb[:],
                    start=(ot == 0),
                    stop=(ot == OT - 1),
                    skip_group_check=True,
                )
            nc.vector.tensor_copy(scores_sb[:, j * SC:(j + 1) * SC], spsum[:])

        # ----- softmax over [1, S] -----
        negm = row_pool.tile([1, 1], F32, tag="negm", name="negm")
        nc.vector.reduce_max(negm[:], scores_sb[:], axis=AX.X, negate=True)
        ew = row_pool.tile([1, S], F32, tag="ew", name="ew")
        nc.scalar.activation(ew[:], scores_sb[:], AF.Exp, bias=negm[:])
        zsum = row_pool.tile([1, 1], F32, tag="z", name="zsum")
        nc.vector.reduce_sum(zsum[:], ew[:], axis=AX.X)
        rz = row_pool.tile([1, 1], F32, tag="rz", name="rz")
        nc.vector.reciprocal(rz[:], zsum[:])
        wrow = row_pool.tile([1, S], F32, tag="wrow", name="wrow")
        nc.vector.tensor_scalar_mul(wrow[:], ew[:], rz[:])
        nc.sync.dma_start(wts_d[b:b + 1, :], wrow[:])

        # ----- context = weights @ keys -----
        wts_sb = []
        for k in range(S // P):
            tp = sm_ps.tile([P, 1], F32, tag="smps", name="wtp")
            nc.tensor.transpose(
                tp[:], wrow[:, k * P:(k + 1) * P], ident[0:1, 0:1]
            )
            wt_sb = wt_pool.tile([P, 1], MMDT, tag="wt", name="wt_sb")
            nc.vector.tensor_copy(wt_sb[:], tp[:])
            wts_sb.append(wt_sb)
        for half in range(2):
            cps = sm_ps.tile([1, SC], F32, tag="smps", name="cps")
            for k in range(S // P):
                nc.tensor.matmul(
                    cps[:],
                    lhsT=wts_sb[k][:],
                    rhs=keys_row[k][:, half * SC:(half + 1) * SC],
                    start=(k == 0),
                    stop=(k == S // P - 1),
                    skip_group_check=True,
                )
            csb = row_pool.tile([1, SC], F32, tag=f"ctx{half}", name=f"csb{half}")
            nc.vector.tensor_copy(csb[:], cps[:])
            nc.sync.dma_start(ctx_d[b:b + 1, half * SC:(half + 1) * SC], csb[:])


_CACHED_NC = {}


def build_nc(repeats: int = 1):
    if repeats in _CACHED_NC:
        return _CACHED_NC[repeats]
    nc = bacc.Bacc(
        "TRN2", target_bir_lowering=False, debug=False, num_devices=NCORES
    )
    io = {
        "keys": nc.dram_tensor("keys", [BC, S, H], MMDT, kind="ExternalInput").ap(),
        "keys_t": nc.dram_tensor("keys_t", [BC, H, S], MMDT, kind="ExternalInput").ap(),
        "query_t": nc.dram_tensor("query_t", [H, BC], F32, kind="ExternalInput").ap(),
        "ua_t": nc.dram_tensor("ua_t", [H, H], MMDT, kind="ExternalInput").ap(),
        "wa_t": nc.dram_tensor("wa_t", [H, H], F32, kind="ExternalInput").ap(),
        "va_t": nc.dram_tensor("va_t", [H, 1], MMDT, kind="ExternalInput").ap(),
        "ab_sum": nc.dram_tensor("ab_sum", [H, 1], F32, kind="ExternalInput").ap(),
        "context": nc.dram_tensor("context", [BC, H], F32, kind="ExternalOutput").ap(),
        "weights": nc.dram_tensor("weights", [BC, S], F32, kind="ExternalOutput").ap(),
    }
    with tile.TileContext(nc) as tc:
        with ExitStack() as ctx:
            build_tile_kernel(ctx, tc, io, repeats=repeats)
    nc.compile()
    _CACHED_NC[repeats] = nc
    return nc


def shard_inputs(inputs):
    """Full inputs -> per-core in_maps.

    Sharding: batch across cores; weights replicated. The shard step also
    produces the transposed layouts / operand dtypes the kernel needs.
    """
    f = lambda x: np.ascontiguousarray(np.asarray(x), dtype=np.float32)
    g = lambda x: np.ascontiguousarray(np.asarray(x, dtype=NPDT))
    query = f(inputs["query"]).reshape(B, H)
    keys = f(inputs["keys"])
    keys_mm = g(keys)                                    # [B, S, H] cast
    keys_t = np.ascontiguousarray(keys.transpose(0, 2, 1).astype(NPDT))
    query_t = np.ascontiguousarray(query.T)              # [H, B]
    ua_t = g(f(inputs["Ua_w"]).T)
    wa_t = np.ascontiguousarray(f(inputs["Wa_w"]).T)
    va_t = g(f(inputs["Va_w"]).reshape(1, H).T)          # [H, 1]
    ab_sum = np.ascontiguousarray(
        (f(inputs["Wa_b"]) + f(inputs["Ua_b"])).reshape(H, 1)
    )
    shared = {"ua_t": ua_t, "wa_t": wa_t, "va_t": va_t, "ab_sum": ab_sum}
    in_maps = []
    for c in range(NCORES):
        sl = slice(c * BC, (c + 1) * BC)
        in_maps.append(
            {
                "keys": np.ascontiguousarray(keys_mm[sl]),
                "keys_t": np.ascontiguousarray(keys_t[sl]),
                "query_t": np.ascontiguousarray(query_t[:, sl]),
                **shared,
            }
        )
    return in_maps


def run(inputs, trace=False):
    """Run on 8 cores; returns ((context, weights), BassKernelResults)."""
    nc = build_nc()
    in_maps = shard_inputs(inputs)
    res = bass_utils.run_bass_kernel_spmd(
        nc, in_maps, list(range(NCORES)), trace=trace
    )
    ctxs = np.stack([r["context"] for r in res.results])  # [8, BC, H]
    wts = np.stack([r["weights"] for r in res.results])  # [8, BC, S]
    context = ctxs.reshape(B, 1, H).astype(np.float32)
    weights = wts.reshape(B, 1, S).astype(np.float32)
    return (context, weights), res


def kernel(**inputs):
    out, _ = run(inputs, trace=False)
    return out


# revision 22
# speedup vs baseline: 1.3143x; 1.1660x over previous
"""Bahdanau attention Trainium2 Bass kernel.

Computation (per batch row b):
    q_proj = query @ Wa_w.T + Wa_b                     # [1, H]
    k_proj = keys  @ Ua_w.T + Ua_b                     # [S, H]
    e      = tanh(q_proj + k_proj)                     # [S, H]
    scores = e @ Va_w.T (+ Va_b, dropped: softmax-invariant)
    weights = softmax(scores)                          # [1, S]
    context = weights @ keys                           # [1, H]

Sharding: data-parallel over batch (32 rows) across 8 cores, 4 rows/core;
weights (Wa/Ua/Va) replicated per core. Host-side sharding re-lays-out the
operands into the layouts the PE needs (keys/Ua/Wa/Va pre-transposed) and
casts the matmul operands to MM_MODE precision, so the device spends no
TensorE cycles on transposes and no DVE cycles on casts.

Per-core dataflow (layout "B": o on partitions so the q_proj+bias add fuses
into the tanh ACT op as a per-partition bias, and scores come out
s-contiguous for the softmax):
  - keysT row-slabs [128(h), 2048(s)] DMA'd straight from DRAM; issue split
    across the SP and Pool sequencers.
  - k_projT[o, s] = UaT.T @ keysT (fp32 accumulate in PSUM).
  - e = tanh(k_projT + bias[o]) on ScalarE.
  - scores[1, s] += VaT[o,1].T @ e accumulated over the 8 o-tiles.
  - softmax on [1, 2048]: DVE reduce_max(negate)/reduce_sum/reciprocal,
    ScalarE exp.
  - weights row transposed to [s,1] columns via tiny PE transposes; context
    [1, h] += wT.T @ keys_nat[s, h] over 16 s-tiles (keys_nat is a second,
    natural-layout DMA stream).

MM_MODE picks the matmul operand dtype:
  "bf16" — fastest: half DMA bytes, ~1e-3..4e-3 rel error.
  "f32r" — fp32 storage, PE-internal rounding: ~1.5e-4 rel error, slower.
"""

import numpy as np
import ml_dtypes
from contextlib import ExitStack

import concourse.bass as bass
import concourse.tile as tile
from concourse import bacc, masks, mybir
from concourse import bass_utils

P = 128
H = 1024
S = 2048
B = 32
NCORES = 8
BC = B // NCORES          # batch rows per core
HT = H // P               # 8 h-tiles
OT = H // P               # 8 o-tiles
SC = 512                  # s-chunk (matmul moving free dim)
NCH = S // SC             # 4 chunks per row

F32 = mybir.dt.float32
F32R = mybir.dt.float32r
BF16 = mybir.dt.bfloat16
AF = mybir.ActivationFunctionType
AX = mybir.AxisListType

MM_MODE = "f32r"
MMDT = BF16 if MM_MODE == "bf16" else F32R
NPDT = ml_dtypes.bfloat16 if MM_MODE == "bf16" else np.float32


def build_tile_kernel(ctx: ExitStack, tc: tile.TileContext, io: dict,
                      repeats: int = 1):
    nc = tc.nc

    # ---- pools ----
    const = ctx.enter_context(tc.tile_pool(name="const", bufs=1))
    wpool = ctx.enter_context(tc.tile_pool(name="wts", bufs=1))
    # natural keys tiles [128, 1024] (context operand)
    keys_pool = ctx.enter_context(tc.tile_pool(name="keys", bufs=16))
    # keysT row-slabs [128(h), 2048(s)]: 8 per row + prefetch slack
    kt_pool = ctx.enter_context(tc.tile_pool(name="kt", bufs=16))
    e_pool = ctx.enter_context(tc.tile_pool(name="e", bufs=8))
    row_pool = ctx.enter_context(tc.tile_pool(name="row", bufs=1))
    wt_pool = ctx.enter_context(tc.tile_pool(name="wt", bufs=20))
    kproj_ps = ctx.enter_context(tc.tile_pool(name="kproj", bufs=4, space="PSUM"))
    sc_ps = ctx.enter_context(tc.tile_pool(name="scps", bufs=2, space="PSUM"))
    sm_ps = ctx.enter_context(tc.tile_pool(name="smps", bufs=2, space="PSUM"))

    ident = const.tile([P, P], F32)
    masks.make_identity(nc, ident[:])

    env = dict(locals())
    for _rep in range(repeats):
        _one_pass(nc, io, env)


def _one_pass(nc, io, env):
    wpool = env["wpool"]; keys_pool = env["keys_pool"]
    kt_pool = env["kt_pool"]; e_pool = env["e_pool"]; row_pool = env["row_pool"]
    wt_pool = env["wt_pool"]; kproj_ps = env["kproj_ps"]
    sc_ps = env["sc_ps"]; sm_ps = env["sm_ps"]
    ident = env["ident"]
    kt_d = io["keys_t"]; k_d = io["keys"]; qt_d = io["query_t"]
    uat_d = io["ua_t"]; wat_d = io["wa_t"]; vat_d = io["va_t"]
    ab_d = io["ab_sum"]
    ctx_d = io["context"]; wts_d = io["weights"]

    # ============== prep: DMA pre-transposed weights ==============
    uat = [wpool.tile([P, H], MMDT, tag=f"uat{ht}", name=f"uat{ht}")
           for ht in range(HT)]
    for ht in range(HT):
        nc.sync.dma_start(uat[ht][:], uat_d[ht * P:(ht + 1) * P, :])
    # WaT tiles (transient, q_proj only): borrow kt-pool slots
    wat = [kt_pool.tile([P, H], F32, tag="kt", name=f"wat{ht}")
           for ht in range(HT)]
    for ht in range(HT):
        nc.gpsimd.dma_start(wat[ht][:], wat_d[ht * P:(ht + 1) * P, :])
    qt = [wpool.tile([P, BC], F32, tag=f"qt{ht}", name=f"qt{ht}")
          for ht in range(HT)]
    for ht in range(HT):
        nc.sync.dma_start(qt[ht][:], qt_d[ht * P:(ht + 1) * P, :])
    vat = [wpool.tile([P, 1], MMDT, tag=f"vat{ot}", name=f"vat{ot}")
           for ot in range(OT)]
    for ot in range(OT):
        nc.sync.dma_start(vat[ot][:], vat_d[ot * P:(ot + 1) * P, :])
    bias_col = [wpool.tile([P, 1], F32, tag=f"bc{ot}", name=f"bc{ot}")
                for ot in range(OT)]
    for ot in range(OT):
        nc.sync.dma_start(bias_col[ot][:], ab_d[ot * P:(ot + 1) * P, :])

    # q_projT[ot] [128, BC] + bias column -> tanh bias vectors
    biases = [wpool.tile([P, BC], F32, tag=f"bias{ot}", name=f"bias{ot}")
              for ot in range(OT)]
    for ot in range(OT):
        pq = sm_ps.tile([P, BC], F32, tag="smps", name="pq")
        for ht in range(HT):
            nc.tensor.matmul(
                pq[:],
                lhsT=wat[ht][:, ot * P:(ot + 1) * P],
                rhs=qt[ht][:],
                start=(ht == 0),
                stop=(ht == HT - 1),
            )
        nc.vector.tensor_scalar_add(biases[ot][:], pq[:], bias_col[ot][:])

    # =====================  main loop over batch rows  =====================
    for b in range(BC):
        scores_sb = row_pool.tile([1, S], F32, tag="scores", name="scores_sb")
        cm4 = row_pool.tile([1, NCH], F32, tag="cm4", name="cm4")
        # keysT row-slabs [128(h), 2048(s)]: one DMA per h-tile, split
        # across two sequencers
        kts = []  # kts[hh][ht] covers s half hh
        for hh in range(2):
            row = []
            for ht in range(HT):
                kt = kt_pool.tile([P, S // 2], MMDT, tag="kt", name="kt")
                nc.sync.dma_start(
                    kt[:],
                    kt_d[b, ht * P:(ht + 1) * P,
                         hh * (S // 2):(hh + 1) * (S // 2)],
                )
                row.append(kt)
            kts.append(row)
        # natural keys tiles for the context matmul
        keys_row = []
        for k in range(S // P):
            knat = keys_pool.tile([P, H], MMDT, tag="keys", name="knat")
            nc.gpsimd.dma_start(knat[:], k_d[b, k * P:(k + 1) * P, :])
            keys_row.append(knat)
        for j in range(NCH):
            spsum = sc_ps.tile([1, SC], F32, tag="scps", name="spsum")
            for ot in range(OT):
                kp = kproj_ps.tile([P, SC], F32, tag="kproj", name="kp")
                for ht in range(HT):
                    nc.tensor.matmul(
                        kp[:],
                        lhsT=uat[ht][:, ot * P:(ot + 1) * P],
                        rhs=kts[j // 2][ht][:, (j % 2) * SC:(j % 2 + 1) * SC],
                        start=(ht == 0),
                        stop=(ht == HT - 1),
                    )
                e_sb = e_pool.tile([P, SC], MMDT, tag="e", name="e_sb")
                nc.scalar.activation(
                    e_sb[:], kp[:], AF.Tanh, bias=biases[ot][:, b:b + 1]
                )
                nc.tensor.matmul(
                    spsum[:],
                    lhsT=vat[ot][:],
                    rhs=e_sb[:],
                    start=(ot == 0),
                    stop=(ot == OT - 1),
                    skip_group_check=True,
                )
            nc.vector.tensor_copy(scores_sb[:, j * SC:(j + 1) * SC], spsum[:])
            # eager per-chunk negated max (hidden under the next chunk's MMs)
            nc.vector.reduce_max(
                cm4[:, j:j + 1], spsum[:], axis=AX.X, negate=True
            )

        # ----- softmax over [1, S] -----
        # -global_max = min of the per-chunk negated maxes (tiny combine)
        negm = row_pool.tile([1, 1], F32, tag="negm", name="negm")
        nc.vector.tensor_reduce(
            negm[:], cm4[:], axis=AX.X, op=mybir.AluOpType.min
        )
        # exp with fused free-dim accumulation -> Z in the same instruction
        ew = row_pool.tile([1, S], F32, tag="ew", name="ew")
        zsum = row_pool.tile([1, 1], F32, tag="z", name="zsum")
        nc.scalar.activation(ew[:], scores_sb[:], AF.Exp, bias=negm[:],
                             accum_out=zsum[:])
        rz = row_pool.tile([1, 1], F32, tag="rz", name="rz")
        nc.vector.reciprocal(rz[:], zsum[:])
        wrow = row_pool.tile([1, S], F32, tag="wrow", name="wrow")
        nc.vector.tensor_scalar_mul(wrow[:], ew[:], rz[:])
        nc.sync.dma_start(wts_d[b:b + 1, :], wrow[:])

        # ----- context = weights @ keys -----
        wts_sb = []
        for k in range(S // P):
            tp = sm_ps.tile([P, 1], F32, tag="smps", name="wtp")
            nc.tensor.transpose(
                tp[:], wrow[:, k * P:(k + 1) * P], ident[0:1, 0:1]
            )
            wt_sb = wt_pool.tile([P, 1], MMDT, tag="wt", name="wt_sb")
            nc.vector.tensor_copy(wt_sb[:], tp[:])
            wts_sb.append(wt_sb)
        for half in range(2):
            cps = sm_ps.tile([1, SC], F32, tag="smps", name="cps")
            for k in range(S // P):
                nc.tensor.matmul(
                    cps[:],
                    lhsT=wts_sb[k][:],
                    rhs=keys_row[k][:, half * SC:(half + 1) * SC],
                    start=(k == 0),
                    stop=(k == S // P - 1),
                    skip_group_check=True,
                )
            csb = row_pool.tile([1, SC], F32, tag=f"ctx{half}", name=f"csb{half}")
            nc.vector.tensor_copy(csb[:], cps[:])
            nc.sync.dma_start(ctx_d[b:b + 1, half * SC:(half + 1) * SC], csb[:])


_CACHED_NC = {}


def build_nc(repeats: int = 1):
    if repeats in _CACHED_NC:
        return _CACHED_NC[repeats]
    nc = bacc.Bacc(
        "TRN2", target_bir_lowering=False, debug=False, num_devices=NCORES
    )
    io = {
        "keys": nc.dram_tensor("keys", [BC, S, H], MMDT, kind="ExternalInput").ap(),
        "keys_t": nc.dram_tensor("keys_t", [BC, H, S], MMDT, kind="ExternalInput").ap(),
        "query_t": nc.dram_tensor("query_t", [H, BC], F32, kind="ExternalInput").ap(),
        "ua_t": nc.dram_tensor("ua_t", [H, H], MMDT, kind="ExternalInput").ap(),
        "wa_t": nc.dram_tensor("wa_t", [H, H], F32, kind="ExternalInput").ap(),
        "va_t": nc.dram_tensor("va_t", [H, 1], MMDT, kind="ExternalInput").ap(),
        "ab_sum": nc.dram_tensor("ab_sum", [H, 1], F32, kind="ExternalInput").ap(),
        "context": nc.dram_tensor("context", [BC, H], F32, kind="ExternalOutput").ap(),
        "weights": nc.dram_tensor("weights", [BC, S], F32, kind="ExternalOutput").ap(),
    }
    with tile.TileContext(nc) as tc:
        with ExitStack() as ctx:
            build_tile_kernel(ctx, tc, io, repeats=repeats)
    nc.compile()
    _CACHED_NC[repeats] = nc
    return nc


def shard_inputs(inputs):
    """Full inputs -> per-core in_maps.

    Sharding: batch across cores; weights replicated. The shard step also
    produces the transposed layouts / operand dtypes the kernel needs.
    """
    f = lambda x: np.ascontiguousarray(np.asarray(x), dtype=np.float32)
    g = lambda x: np.ascontiguousarray(np.asarray(x, dtype=NPDT))
    query = f(inputs["query"]).reshape(B, H)
    keys = f(inputs["keys"])
    keys_mm = g(keys)                                    # [B, S, H] cast
    keys_t = np.ascontiguousarray(keys.transpose(0, 2, 1).astype(NPDT))
    query_t = np.ascontiguousarray(query.T)              # [H, B]
    ua_t = g(f(inputs["Ua_w"]).T)
    wa_t = np.ascontiguousarray(f(inputs["Wa_w"]).T)
    va_t = g(f(inputs["Va_w"]).reshape(1, H).T)          # [H, 1]
    ab_sum = np.ascontiguousarray(
        (f(inputs["Wa_b"]) + f(inputs["Ua_b"])).reshape(H, 1)
    )
    shared = {"ua_t": ua_t, "wa_t": wa_t, "va_t": va_t, "ab_sum": ab_sum}
    in_maps = []
    for c in range(NCORES):
        sl = slice(c * BC, (c + 1) * BC)
        in_maps.append(
            {
                "keys": np.ascontiguousarray(keys_mm[sl]),
                "keys_t": np.ascontiguousarray(keys_t[sl]),
                "query_t": np.ascontiguousarray(query_t[:, sl]),
                **shared,
            }
        )
    return in_maps


def run(inputs, trace=False):
    """Run on 8 cores; returns ((context, weights), BassKernelResults)."""
    nc = build_nc()
    in_maps = shard_inputs(inputs)
    res = bass_utils.run_bass_kernel_spmd(
        nc, in_maps, list(range(NCORES)), trace=trace
    )
    ctxs = np.stack([r["context"] for r in res.results])  # [8, BC, H]
    wts = np.stack([r["weights"] for r in res.results])  # [8, BC, S]
    context = ctxs.reshape(B, 1, H).astype(np.float32)
    weights = wts.reshape(B, 1, S).astype(np.float32)
    return (context, weights), res


def kernel(**inputs):
    out, _ = run(inputs, trace=False)
    return out
